# revision 5
# baseline (speedup 1.0000x reference)
"""NeRF MLP (131072 x 90 -> 131072 x 4) on 8 Trainium2 NeuronCores — v6.

Data parallel over rays (16384 rows/core), feature-major on-chip layout,
weights stationary per matmul, batch tile N=512, groups of G=4 batch
tiles emitted batch-major (weight reused across the group before
switching).

v3 vs v2:
- Consolidated DMAs: 4 input DMAs + 1 output DMA per group (was 16+8),
  biases in one blob DMA (was 12). Cuts SP dispatch (~790ns each) and
  the kernel-start serialization stall (~19us in sim).
- DMA issue order: first two groups' inputs first, then weights in
  first-use order interleaved with remaining groups' inputs.
- Drain engine balance: strict ACT/DVE round-robin over ALL PSUM->SBUF
  drains (v2 sent every bias-only drain to DVE, making DVE the
  near-bottleneck at 61% busy vs ACT 41%).
"""

import numpy as np

import concourse.bass as bass
import concourse.mybir as mybir
import concourse.tile as tile
from concourse import bacc, bass_utils

POS, VIEW = 63, 27
NTOT = 131072
NCORES = 8
NCORE = NTOT // NCORES  # 16384
TN = 512
P = 128
F32 = mybir.dt.float32
F32R = mybir.dt.float32r
AF = mybir.ActivationFunctionType
ALU = mybir.AluOpType

G = 4
WDT = F32R

WSPEC = {
    "d1w": [P, P],
    "d2w": [P, 2, 256],
    "d3w": [P, 2, 256],
    "d4w": [P, 2, 256],
    "e1pw": [P, P],
    "e1hw": [P, 2, 256],
    "e2w": [P, 2, 256],
    "e3w": [P, 2, 256],
    "e4w": [P, 2, 256],
    "e5fw": [P, 2, 256],
    "e5dw": [P, 2, 1],
    "c1vw": [64, P],
    "c1fw": [P, 2, 256],
    "c2w": [P, 2, 3],
}
# bias blob layout: 10 dense [P,2] biases at columns 2i:2i+2, then
# e5db at [0:1, 20:21], c2b at [0:3, 21:22]
BLAYOUT = ["d1b", "d2b", "d3b", "d4b", "e1b", "e2b", "e3b", "e4b",
           "e5fb", "c1b"]
BCOLS = 22


def build_nc(ncore=NCORE, repeat=1):
    nt = ncore // TN
    ng = nt // G
    nc = bacc.Bacc(
        "TRN2", target_bir_lowering=False, debug=False, enable_asserts=False
    )

    xT = nc.dram_tensor("xT", [POS + VIEW, ncore], WDT, kind="ExternalInput")
    outT = nc.dram_tensor("outT", [4, ncore], F32, kind="ExternalOutput")
    dram = {k: nc.dram_tensor(k, v, WDT, kind="ExternalInput")
            for k, v in WSPEC.items()}
    bblob = nc.dram_tensor("bblob", [P, BCOLS], F32, kind="ExternalInput")

    with tile.TileContext(nc) as tc:
        with (
            tc.tile_pool(name="w", bufs=1) as wpool,
            tc.tile_pool(name="act", bufs=2 * G) as apool,
            tc.tile_pool(name="xin", bufs=2) as xpool,
            tc.tile_pool(name="out", bufs=2) as opool,
            tc.tile_pool(name="psum", bufs=8, space="PSUM") as pspool,
        ):
            # --- input prefetch: pos and view fetched separately so the
            # view (first needed at c1, late in a group) never delays the
            # first matmuls of the group
            def fetch_pos(g):
                col0 = g * G * TN
                cw = G * TN
                pg = xpool.tile([P, G, TN], WDT, tag="posg", name=f"posg_{g}")
                src_p = xT[:POS, col0:col0 + cw]
                nc.sync.dma_start(pg[0:POS, :, :], src_p)
                nc.sync.dma_start(pg[64:64 + POS, :, :], src_p)
                return pg

            def fetch_view(g):
                col0 = g * G * TN
                cw = G * TN
                vg = xpool.tile([64, G, TN], WDT, tag="viewg",
                                name=f"viewg_{g}")
                src_v = xT[POS:, col0:col0 + cw]
                nc.sync.dma_start(vg[0:VIEW, :, :], src_v)
                nc.sync.dma_start(vg[32:32 + VIEW, :, :], src_v)
                return vg

            sb = {}

            def load_w(keys):
                for k in keys:
                    t = wpool.tile(WSPEC[k], WDT, tag=k, name=f"sb_{k}")
                    nc.sync.dma_start(t[:], dram[k][:])
                    sb[k] = t

            # DMA issue order = SP execution order: everything the head of
            # the pipeline needs first, the rest interleaved behind it
            load_w(["d1w"])
            sbb = wpool.tile([P, BCOLS], F32, tag="bb", name="sb_bb")
            nc.sync.dma_start(sbb[:], bblob[:])
            bias = {k: sbb[:, 2 * i:2 * i + 2] for i, k in enumerate(BLAYOUT)}
            bias["e5db"] = sbb[0:1, 20:21]
            bias["c2b"] = sbb[0:3, 21:22]
            pending = {}
            if repeat == 1:
                # weights interleaved with the first fetches so the first
                # matmul starts ~7us in (pure weight-first order: ~20us)
                pending[0] = [fetch_pos(0), None]
                load_w(["d2w", "d3w", "d4w"])
                pending[0][1] = fetch_view(0)
                if ng > 1:
                    pending[1] = [fetch_pos(1), None]
                load_w(["e1pw", "e1hw", "e2w", "e3w", "e4w"])
                if ng > 1:
                    pending[1][1] = fetch_view(1)
                load_w(["e5fw", "e5dw", "c1vw", "c1fw", "c2w"])
            else:
                # repeat>1 wraps emit() in a hardware loop: all prefetch
                # priming must live inside the loop body or the tile-slot
                # reuse across iterations deadlocks the scheduler
                load_w(["d2w", "d3w", "d4w", "e1pw", "e1hw", "e2w", "e3w",
                        "e4w", "e5fw", "e5dw", "c1vw", "c1fw", "c2w"])

            eng_ctr = [0]

            def store(ps_ap, dest_ap, bias_ap, relu):
                use_act = (eng_ctr[0] % 2) == 0
                eng_ctr[0] += 1
                if use_act:
                    # Identity (not Copy) accepts an AP bias; both share
                    # relu's activation-table set, so no table reloads
                    nc.scalar.activation(
                        dest_ap, ps_ap, AF.Relu if relu else AF.Identity,
                        bias=bias_ap)
                elif relu:
                    nc.vector.tensor_scalar(
                        dest_ap, ps_ap, bias_ap, 0.0, op0=ALU.add, op1=ALU.max
                    )
                else:
                    nc.vector.tensor_scalar_add(dest_ap, ps_ap, bias_ap)

            def layer256(wk, rhs_of, bk, outs, relu, g, nm=2, nk=2):
                # drain each tile's half as soon as its last k-chunk lands,
                # not after the whole m-half sweep: the next layer's first
                # matmuls wait on these drains
                pss = [[None] * nm for _ in range(len(outs))]
                for m in range(nm):
                    for k in range(nk):
                        for ti in range(len(outs)):
                            if k == 0:
                                ps = pspool.tile([P, TN], F32, tag="ps",
                                                 name=f"ps_{g}_{m}_{ti}")
                                pss[ti][m] = ps
                            nc.tensor.matmul(
                                pss[ti][m][:], wk(k, m), rhs_of(ti, k),
                                start=(k == 0), stop=(k == nk - 1),
                            )
                            if k == nk - 1:
                                store(pss[ti][m][:], outs[ti][:, m, :],
                                      bk[:, m:m + 1], relu)

            def packed_open(wpk, rhs_of2, n, lo_hi, g, tag, close=False):
                pss = []
                for ti in range(n):
                    row = []
                    for m, (lo, hi) in enumerate(lo_hi):
                        ps = pspool.tile([P, TN], F32, tag="ps",
                                         name=f"pp_{g}_{tag}_{ti}_{m}")
                        nc.tensor.matmul(
                            ps[:], wpk[lo:hi, :], rhs_of2(ti, lo, hi),
                            start=True, stop=close,
                            skip_group_check=not close,
                        )
                        row.append(ps)
                    pss.append(row)
                return pss

            def packed_close(pss, whk, rhs_of, bk, outs, relu, nk=2):
                for m in range(2):
                    for k in range(nk):
                        for ti in range(len(outs)):
                            nc.tensor.matmul(
                                pss[ti][m][:], whk(k, m), rhs_of(ti, k),
                                start=False, stop=(k == nk - 1),
                                skip_group_check=True,
                            )
                            if k == nk - 1:
                                store(pss[ti][m][:], outs[ti][:, m, :],
                                      bk[:, m:m + 1], relu)

            def do_group(g):
                pg, vg = pending.pop(g)
                if g + 2 < ng:
                    pending[g + 2] = [fetch_pos(g + 2), fetch_view(g + 2)]
                n = G
                tl = list(range(g * G, (g + 1) * G))

                ab = [0]

                def ht(nm):
                    tag = "actA" if ab[0] % 2 == 0 else "actB"
                    ab[0] += 1
                    return [apool.tile([P, 2, TN], WDT, tag=tag,
                                       name=f"{nm}_{t}") for t in tl]

                def prhs(ti, lo, hi):
                    return pg[lo:hi, ti, :]

                def vrhs(ti, lo, hi):
                    return vg[lo:hi, ti, :]

                h1 = ht("h1")
                pss = packed_open(sb["d1w"], prhs, n,
                                  ((0, POS), (64, 64 + POS)), g, "d1",
                                  close=True)
                for ti in range(n):
                    for m in range(2):
                        store(pss[ti][m][:], h1[ti][:, m, :],
                              bias["d1b"][:, m:m + 1], True)

                def w2(name):
                    return lambda k, m: sb[name][:, k, m * P:(m + 1) * P]

                def rhs(hs):
                    return lambda ti, k: hs[ti][:, k, :]

                h2 = ht("h2")
                layer256(w2("d2w"), rhs(h1), bias["d2b"], h2, True, g)
                h3 = ht("h3")
                layer256(w2("d3w"), rhs(h2), bias["d3b"], h3, True, g)
                h4 = ht("h4")
                layer256(w2("d4w"), rhs(h3), bias["d4b"], h4, True, g)

                g1 = ht("g1")
                pss = packed_open(sb["e1pw"], prhs, n,
                                  ((0, POS), (64, 64 + POS)), g, "e1")
                packed_close(pss, w2("e1hw"), rhs(h4), bias["e1b"], g1, True)

                g2 = ht("g2")
                layer256(w2("e2w"), rhs(g1), bias["e2b"], g2, True, g)
                g3 = ht("g3")
                layer256(w2("e3w"), rhs(g2), bias["e3b"], g3, True, g)
                g4 = ht("g4")
                layer256(w2("e4w"), rhs(g3), bias["e4b"], g4, True, g)

                f = ht("f")
                layer256(w2("e5fw"), rhs(g4), bias["e5fb"], f, False, g)

                rgbg = opool.tile([3, G, TN], F32, tag="rgbg",
                                  name=f"rgbg_{g}")
                densg = opool.tile([1, G, TN], F32, tag="densg",
                                   name=f"densg_{g}")

                psd = [pspool.tile([P, TN], F32, tag="ps",
                                   name=f"psd_{g}_{ti}") for ti in range(n)]
                for k in (0, 1):
                    for ti in range(n):
                        nc.tensor.matmul(
                            psd[ti][0:1, :], sb["e5dw"][:, k, :],
                            g4[ti][:, k, :], start=(k == 0), stop=(k == 1),
                        )
                for ti in range(n):
                    store(psd[ti][0:1, :], densg[:, ti, :], bias["e5db"],
                          False)

                cc = ht("cc")
                pss = packed_open(sb["c1vw"], vrhs, n,
                                  ((0, VIEW), (32, 32 + VIEW)), g, "c1")
                packed_close(pss, w2("c1fw"), rhs(f), bias["c1b"], cc, True)

                psc = [pspool.tile([P, TN], F32, tag="ps",
                                   name=f"psc_{g}_{ti}") for ti in range(n)]
                for k in (0, 1):
                    for ti in range(n):
                        nc.tensor.matmul(
                            psc[ti][0:3, :], sb["c2w"][:, k, :],
                            cc[ti][:, k, :], start=(k == 0), stop=(k == 1),
                        )
                for ti in range(n):
                    store(psc[ti][0:3, :], rgbg[:, ti, :], bias["c2b"], False)

                col0 = g * G * TN
                nc.sync.dma_start(outT[0:3, col0:col0 + G * TN], rgbg[:])
                nc.sync.dma_start(outT[3:4, col0:col0 + G * TN], densg[:])

            def emit():
                if not pending:
                    pending[0] = [fetch_pos(0), fetch_view(0)]
                    if ng > 1:
                        pending[1] = [fetch_pos(1), fetch_view(1)]
                for g in range(ng):
                    do_group(g)

            if repeat == 1:
                emit()
            else:
                with tc.For_i(0, repeat, 1):
                    emit()
    nc.compile()
    return nc


def shard_inputs(inputs, ncore=NCORE, ncores=NCORES):
    x = np.asarray(inputs["x"], dtype=np.float32)

    def t2(w):
        return np.ascontiguousarray(
            np.asarray(w, np.float32).reshape(2, P, -1).transpose(1, 0, 2))

    def tb(b):
        return np.asarray(b, np.float32).reshape(2, P).T

    def pack_rows(w, off, rows):
        out = np.zeros((rows, P), np.float32)
        out[:w.shape[0], :] = w[:, :P]
        out[off:off + w.shape[0], :] = w[:, P:]
        return np.ascontiguousarray(out)

    i = {k: np.asarray(v, np.float32) for k, v in inputs.items()}
    shared = {
        "d1w": pack_rows(i["d1_w"], 64, P),
        "d2w": t2(i["d2_w"]), "d3w": t2(i["d3_w"]), "d4w": t2(i["d4_w"]),
        "e1pw": pack_rows(i["e1_w"][:POS], 64, P),
        "e1hw": t2(i["e1_w"][POS:]),
        "e2w": t2(i["e2_w"]), "e3w": t2(i["e3_w"]), "e4w": t2(i["e4_w"]),
        "e5fw": t2(i["e5_w"][:, 1:]),
        "e5dw": t2(i["e5_w"][:, :1]),
        "c1vw": pack_rows(i["c1_w"][:VIEW], 32, 64),
        "c1fw": t2(i["c1_w"][VIEW:]),
        "c2w": t2(i["c2_w"]),
    }
    bb = np.zeros((P, BCOLS), np.float32)
    bvals = {
        "d1b": tb(i["d1_b"]), "d2b": tb(i["d2_b"]), "d3b": tb(i["d3_b"]),
        "d4b": tb(i["d4_b"]), "e1b": tb(i["e1_b"]), "e2b": tb(i["e2_b"]),
        "e3b": tb(i["e3_b"]), "e4b": tb(i["e4_b"]),
        "e5fb": tb(i["e5_b"][1:]), "c1b": tb(i["c1_b"]),
    }
    for j, k in enumerate(BLAYOUT):
        bb[:, 2 * j:2 * j + 2] = bvals[k]
    bb[0, 20] = i["e5_b"][0]
    bb[0:3, 21] = i["c2_b"]
    shared["bblob"] = np.ascontiguousarray(bb)
    in_maps = []
    for c in range(ncores):
        xc = np.ascontiguousarray(x[c * ncore:(c + 1) * ncore, :].T)
        in_maps.append({"xT": xc, **shared})
    return in_maps


LAST_RESULTS = None
_NC_CACHE = {}


def _run(inputs, repeat=1):
    global LAST_RESULTS
    key = (NCORE, repeat)
    if key not in _NC_CACHE:
        _NC_CACHE[key] = build_nc(NCORE, repeat)
    nc = _NC_CACHE[key]
    in_maps = shard_inputs(inputs)
    res = bass_utils.run_bass_kernel_spmd(nc, in_maps,
                                          core_ids=list(range(NCORES)))
    LAST_RESULTS = res
    out = np.concatenate([res.results[c]["outT"] for c in range(NCORES)],
                         axis=1)
    return np.ascontiguousarray(out.T).astype(np.float32, copy=False)


def kernel(**inputs):
    return _run(inputs, 1)
